# revision 1
# baseline (speedup 1.0000x reference)
"""Locally-connected network (28x28 -> lc3x3 -> lc3x3 -> fc10) on 8 TRN2 cores.

The whole reference network is linear (two locally-connected layers + FC, no
activations), so on the host we fold it into a single affine map
    out[b, :] = x[b, :784] @ M + c          (M: [784, 10], c: [10])
computed in float64. The device kernel is then a pure data-parallel,
memory-bound matmul: each of the 8 cores streams its 1024-sample shard of x
(transposed to pixel-major [784, 1024] on host) and does 7 accumulating
K-tile matmuls into PSUM, adds the bias via ScalarE, and writes [10, 1024].
"""

import numpy as np

import concourse.bass as bass
import concourse.tile as tile
from concourse import bacc, mybir
from concourse.bass_utils import run_bass_kernel_spmd

N_CORES = 8
B = 8192
B_SHARD = B // N_CORES          # 1024
PIX = 784                       # 28*28
KP = 112                        # K-tile partition count; 784 = 7 * 112
NKT = PIX // KP                 # 7
NCHUNK = 2                      # batch chunks of 512 (PSUM bank = 512 f32)
CH = B_SHARD // NCHUNK          # 512
NOUT = 10
MW_COLS = 128                   # padded free dim (512B rows); cols 0..69 = M, col 70 = bias


def _lc_dense(w, H, W_, oh, ow):
    """Dense [H*W_, oh*ow] matrix of one 3x3 locally-connected layer."""
    w = np.asarray(w, np.float64).reshape(oh, ow, 9)
    M = np.zeros((H * W_, oh * ow), np.float64)
    ox, oy = np.meshgrid(np.arange(oh), np.arange(ow), indexing="ij")
    col = (ox * ow + oy).ravel()
    for i in range(3):
        for j in range(3):
            row = ((ox + i) * W_ + (oy + j)).ravel()
            M[row, col] += w[:, :, i * 3 + j].ravel()
    return M


def _fold(w1, b1, w2, b2, fc_w, fc_b):
    W1 = _lc_dense(w1, 28, 28, 26, 26)          # [784, 676]
    W2 = _lc_dense(w2, 26, 26, 24, 24)          # [676, 576]
    fcw = np.asarray(fc_w, np.float64)          # [10, 576]
    M = W1 @ W2 @ fcw.T                         # [784, 10]
    c = (
        np.asarray(b1, np.float64).reshape(-1) @ W2
        + np.asarray(b2, np.float64).reshape(-1)
    ) @ fcw.T + np.asarray(fc_b, np.float64)    # [10]
    return M.astype(np.float32), c.astype(np.float32)


def _build_bass():
    nc = bacc.Bacc("TRN2", target_bir_lowering=False, debug=False)
    xt = nc.declare_dram_parameter("xt", [KP, NKT, B_SHARD], mybir.dt.float32, isOutput=False)
    mw = nc.declare_dram_parameter("mw", [KP, MW_COLS], mybir.dt.float32, isOutput=False)
    out = nc.declare_dram_parameter("out", [NOUT, B_SHARD], mybir.dt.float32, isOutput=True)

    with tile.TileContext(nc) as tc:
        with (
            tc.tile_pool(name="wp", bufs=1) as wp,
            tc.tile_pool(name="xp", bufs=NKT) as xp,
            tc.tile_pool(name="pp", bufs=NCHUNK, space="PSUM") as pp,
            tc.tile_pool(name="op", bufs=NCHUNK) as op,
        ):
            m_sb = wp.tile([KP, MW_COLS], mybir.dt.float32)
            nc.sync.dma_start(m_sb[:], mw[:])

            # TRN2 LDWEIGHTS lowering allows a single sync wait; a matmul
            # whose operands arrive via two DMA lanes fails codegen ("too
            # many sync wait commands"). Absorb the m_sb wait on PE with a
            # throwaway matmul that only reads m_sb, so every real matmul
            # waits on at most its own x-tile lane.
            warm = pp.tile([NOUT, 1], mybir.dt.float32)
            nc.tensor.matmul(
                warm[:], m_sb[:, 0:NOUT], m_sb[:, 0:1], start=True, stop=True
            )
            # Same single-wait constraint on ScalarE: the bias-add below reads
            # both PSUM (PE sem) and m_sb (DMA lane); touch m_sb here so the
            # real activation only waits on the PE sem.
            scratch = op.tile([1, 1], mybir.dt.float32)
            nc.scalar.copy(scratch[:], m_sb[0:1, 0:1])

            # Keep total DMA count <= 8 so no DMAHW semaphore lane is reused
            # (lane reuse adds a second sync wait to a DMA, which TRN2
            # codegen rejects). 4 x-loads + m_sb + 1 output = 6 lanes.
            # x is packed [KP, NKT, B] on host so each partition reads
            # contiguous 8KB per 2-k-tile group; loads alternate between the
            # two HWDGE rings (sync / scalar) to double descriptor feed rate.
            groups = [(0, 2), (2, 2), (4, 2), (6, 1)]  # (first kt, n k-tiles)
            rings = [nc.sync, nc.sync, nc.sync, nc.sync]
            xts = [None] * NKT
            for (k0, nk), ring in zip(groups, rings):
                t = xp.tile([KP, nk, B_SHARD], mybir.dt.float32)
                ring.dma_start(t[:], xt[:, k0 : k0 + nk, :])
                for j in range(nk):
                    xts[k0 + j] = (t, j)

            o = op.tile([NOUT, B_SHARD], mybir.dt.float32)
            for ch in range(NCHUNK):
                ps = pp.tile([NOUT, CH], mybir.dt.float32)
                for kt in range(NKT):
                    t, j = xts[kt]
                    nc.tensor.matmul(
                        ps[:],
                        m_sb[:, kt * NOUT : (kt + 1) * NOUT],
                        t[:, j, ch * CH : (ch + 1) * CH],
                        start=(kt == 0),
                        stop=(kt == NKT - 1),
                    )
                nc.scalar.activation(
                    o[:, ch * CH : (ch + 1) * CH],
                    ps[:],
                    mybir.ActivationFunctionType.Identity,
                    bias=m_sb[0:NOUT, 70:71],
                )
            nc.sync.dma_start(out[:], o[:])
    nc.finalize()
    return nc


def _run(inputs, trace=False, trace_cores=None):
    x = np.asarray(inputs["x"], np.float32)
    M, c = _fold(
        inputs["w1"], inputs["b1"], inputs["w2"], inputs["b2"],
        inputs["fc_w"], inputs["fc_b"],
    )
    mp = np.zeros((KP, MW_COLS), np.float32)
    for kt in range(NKT):
        mp[:, kt * NOUT : (kt + 1) * NOUT] = M[kt * KP : (kt + 1) * KP]
    mp[0:NOUT, 70] = c

    # Pack per-core shard to [KP, NKT, B_SHARD]: xt[p, kt, b] = x[b, kt*KP+p],
    # so every partition's k-tile group is one contiguous DRAM read.
    xr = x.reshape(B, PIX)
    in_maps = [
        {
            "xt": np.ascontiguousarray(
                xr[i * B_SHARD : (i + 1) * B_SHARD]
                .reshape(B_SHARD, NKT, KP)
                .transpose(2, 1, 0)
            ),
            "mw": mp,
        }
        for i in range(N_CORES)
    ]

    nc = _build_bass()
    res = run_bass_kernel_spmd(
        nc,
        in_maps,
        list(range(N_CORES)),
        trace=trace,
        trace_cores=trace_cores,
    )
    out = np.concatenate(
        [np.asarray(res.results[i]["out"]).T for i in range(N_CORES)], axis=0
    ).astype(np.float32)
    return out, res


def kernel(**inputs) -> np.ndarray:
    out, _ = _run(inputs, trace=False)
    return out



# revision 2
# speedup vs baseline: 1.4177x; 1.4177x over previous
"""Locally-connected network (28x28 -> lc3x3 -> lc3x3 -> fc10) on 8 TRN2 cores.

The whole reference network is linear (two locally-connected layers + FC, no
activations), so on the host we fold it into a single affine map
    out[b, :] = x[b, :784] @ M + c          (M: [784, 10], c: [10])
computed in float64. The device kernel is then a pure data-parallel,
memory-bound matmul: each of the 8 cores streams its 1024-sample shard of x
(transposed to pixel-major [784, 1024] on host, cast to bf16 — rel err ~3e-3,
well under the 2e-2 gate) and does 7 accumulating K-tile matmuls into PSUM,
adds the bias via ScalarE, and writes [10, 1024] fp32. bf16 halves the HBM
traffic vs fp32 AND runs the PE at 1 cycle/row instead of 4.
"""

import numpy as np
import ml_dtypes

import concourse.bass as bass
import concourse.tile as tile
from concourse import bacc, mybir
from concourse.bass_utils import run_bass_kernel_spmd

N_CORES = 8
B = 8192
B_SHARD = B // N_CORES          # 1024
PIX = 784                       # 28*28
KP = 112                        # K-tile partition count; 784 = 7 * 112
NKT = PIX // KP                 # 7
NCHUNK = 2                      # batch chunks of 512 (PSUM bank = 512 f32)
CH = B_SHARD // NCHUNK          # 512
NOUT = 10
MW_COLS = 128                   # padded free dim; cols 0..69 = M, col 70 = bias


def _lc_dense(w, H, W_, oh, ow):
    """Dense [H*W_, oh*ow] matrix of one 3x3 locally-connected layer."""
    w = np.asarray(w, np.float64).reshape(oh, ow, 9)
    M = np.zeros((H * W_, oh * ow), np.float64)
    ox, oy = np.meshgrid(np.arange(oh), np.arange(ow), indexing="ij")
    col = (ox * ow + oy).ravel()
    for i in range(3):
        for j in range(3):
            row = ((ox + i) * W_ + (oy + j)).ravel()
            M[row, col] += w[:, :, i * 3 + j].ravel()
    return M


def _fold(w1, b1, w2, b2, fc_w, fc_b):
    W1 = _lc_dense(w1, 28, 28, 26, 26)          # [784, 676]
    W2 = _lc_dense(w2, 26, 26, 24, 24)          # [676, 576]
    fcw = np.asarray(fc_w, np.float64)          # [10, 576]
    M = W1 @ W2 @ fcw.T                         # [784, 10]
    c = (
        np.asarray(b1, np.float64).reshape(-1) @ W2
        + np.asarray(b2, np.float64).reshape(-1)
    ) @ fcw.T + np.asarray(fc_b, np.float64)    # [10]
    return M.astype(np.float32), c.astype(np.float32)


def _build_bass():
    nc = bacc.Bacc("TRN2", target_bir_lowering=False, debug=False)
    xt = nc.declare_dram_parameter("xt", [KP, NKT, B_SHARD], mybir.dt.bfloat16, isOutput=False)
    mw = nc.declare_dram_parameter("mw", [KP, MW_COLS], mybir.dt.bfloat16, isOutput=False)
    out = nc.declare_dram_parameter("out", [NOUT, B_SHARD], mybir.dt.float32, isOutput=True)

    with tile.TileContext(nc) as tc:
        with (
            tc.tile_pool(name="wp", bufs=1) as wp,
            tc.tile_pool(name="xp", bufs=NKT) as xp,
            tc.tile_pool(name="pp", bufs=NCHUNK, space="PSUM") as pp,
            tc.tile_pool(name="op", bufs=NCHUNK) as op,
        ):
            m_sb = wp.tile([KP, MW_COLS], mybir.dt.bfloat16)
            nc.sync.dma_start(m_sb[:], mw[:])

            # TRN2 LDWEIGHTS lowering allows a single sync wait; a matmul
            # whose operands arrive via two DMA lanes fails codegen ("too
            # many sync wait commands"). Absorb the m_sb wait on PE with a
            # throwaway matmul that only reads m_sb, so every real matmul
            # waits on at most its own x-tile lane.
            warm = pp.tile([NOUT, 1], mybir.dt.float32)
            nc.tensor.matmul(
                warm[:], m_sb[:, 0:NOUT], m_sb[:, 0:1], start=True, stop=True
            )
            # Same single-wait constraint on ScalarE: the bias-add below reads
            # both PSUM (PE sem) and m_sb (DMA lane); touch m_sb here so the
            # real activation only waits on the PE sem.
            scratch = op.tile([1, 1], mybir.dt.float32)
            nc.scalar.copy(scratch[:], m_sb[0:1, 0:1])

            # Keep total DMA count <= 8 so no DMAHW semaphore lane is reused
            # (lane reuse adds a second sync wait to a DMA, which TRN2
            # codegen rejects). 4 x-loads + m_sb + 2 output stores = 7 lanes.
            # x is packed [KP, NKT, B] on host so each partition reads
            # contiguous 4KB per 2-k-tile group.
            groups = [(0, 2), (2, 2), (4, 2), (6, 1)]  # (first kt, n k-tiles)
            xts = [None] * NKT
            for k0, nk in groups:
                t = xp.tile([KP, nk, B_SHARD], mybir.dt.bfloat16)
                nc.sync.dma_start(t[:], xt[:, k0 : k0 + nk, :])
                for j in range(nk):
                    xts[k0 + j] = (t, j)

            o = op.tile([NOUT, B_SHARD], mybir.dt.float32)
            for ch in range(NCHUNK):
                ps = pp.tile([NOUT, CH], mybir.dt.float32)
                for kt in range(NKT):
                    t, j = xts[kt]
                    nc.tensor.matmul(
                        ps[:],
                        m_sb[:, kt * NOUT : (kt + 1) * NOUT],
                        t[:, j, ch * CH : (ch + 1) * CH],
                        start=(kt == 0),
                        stop=(kt == NKT - 1),
                    )
                nc.scalar.activation(
                    o[:, ch * CH : (ch + 1) * CH],
                    ps[:],
                    mybir.ActivationFunctionType.Identity,
                    bias=m_sb[0:NOUT, 70:71],
                )
                # store each chunk as soon as its bias-add lands so the
                # final store's completion latency overlaps chunk 1 compute
                nc.sync.dma_start(
                    out[:, ch * CH : (ch + 1) * CH], o[:, ch * CH : (ch + 1) * CH]
                )
    nc.finalize()
    return nc


def _run(inputs, trace=False, trace_cores=None):
    x = np.asarray(inputs["x"], np.float32)
    M, c = _fold(
        inputs["w1"], inputs["b1"], inputs["w2"], inputs["b2"],
        inputs["fc_w"], inputs["fc_b"],
    )
    mp = np.zeros((KP, MW_COLS), np.float32)
    for kt in range(NKT):
        mp[:, kt * NOUT : (kt + 1) * NOUT] = M[kt * KP : (kt + 1) * KP]
    mp[0:NOUT, 70] = c
    mp = mp.astype(ml_dtypes.bfloat16)

    # Pack per-core shard to [KP, NKT, B_SHARD]: xt[p, kt, b] = x[b, kt*KP+p],
    # so every partition's k-tile group is one contiguous DRAM read.
    xr = x.reshape(B, PIX)
    in_maps = [
        {
            "xt": np.ascontiguousarray(
                xr[i * B_SHARD : (i + 1) * B_SHARD]
                .reshape(B_SHARD, NKT, KP)
                .transpose(2, 1, 0)
            ).astype(ml_dtypes.bfloat16),
            "mw": mp,
        }
        for i in range(N_CORES)
    ]

    nc = _build_bass()
    res = run_bass_kernel_spmd(
        nc,
        in_maps,
        list(range(N_CORES)),
        trace=trace,
        trace_cores=trace_cores,
    )
    out = np.concatenate(
        [np.asarray(res.results[i]["out"]).T for i in range(N_CORES)], axis=0
    ).astype(np.float32)
    return out, res


def kernel(**inputs) -> np.ndarray:
    out, _ = _run(inputs, trace=False)
    return out
